# revision 1
# baseline (speedup 1.0000x reference)
"""NeighborRoutingConv (GAT-style multi-head edge-softmax message passing) on 8 trn2 cores.

Strategy (v3, dma_gather edition):
  - Host folds attn into the weight matrix: a[n,k] = sum_i h[n,i]*c[k,i] with
    c[k,:] = sum_j attn[k,j] * W[k*32+j, :].  One matmul per node tile emits
    whaug[n] = [ (h @ W.T)(256) ; a(8) ; pad(to 320) ].
  - Phase 1 (replicated on every core): compute whaug for all N nodes into
    core-local DRAM (320 f32 row stride for dma_gather's 256B-granularity).
  - Phase 2 (dst-sharded): edges grouped by 128-node destination blocks;
    blocks bin-packed into (core, slot) pairs so per-slot chunk counts are
    compile-time constants shared by all cores (SPMD).  Edges of a block are
    split by src < HALF into segment A/B (dma_gather idx is int16).  Each
    segment ends with a "header" chunk whose 128 entries gather the block's
    own 128 dst rows (the segment matching the block's half; the other
    segment's header gathers row 0 junk).  Per block:
      * dma_gather whaug[src] rows (320 f32) per segment -> M0 [128, nch, 320]
      * a_dst[128,8] = hdrA.aux*wA + hdrB.aux*wB  (host-provided 0/1 weights)
      * sel[e, ci, d] = (dcol[e,ci]==d)  batched is_equal (one-hot masks)
      * per chunk: PE-transpose sel_ci -> S (d-major); a_dst_e = S.T @ a_dst
      * e_exp = exp(leakyrelu(a_src + a_dst_e)) batched, into M0 aux cols
      * msgs *= bcast(e_exp); per chunk one PE matmul accumulates
        [segment_sum(msgs) ; segment_sum(e_exp)] into PSUM [128, 264]
      * out_block = psum[:, :256] * bcast(1/(e_sum+eps)) -> DMA out.
  Softmax max-subtraction is skipped (mathematically identical; |a| <~ 10 so
  e_exp stays in fp32 range).
"""

import math
from contextlib import ExitStack

import numpy as np

P = 128
IN_DIM = 256
OUT_DIM = 256
K = 8
DK = 32
ROW = 320  # whaug row stride (f32): Wh(256) | a(8) | pad
AUX = OUT_DIM  # aux column offset
RHS = OUT_DIM + K  # 264 — matmul rhs width (msgs + e_exp)
NEG_SLOPE = 0.2
N_CORES = 8
SUPER = 4  # node tiles per phase-1 iteration (512 nodes)


def _ceil_div(a, b):
    return (a + b - 1) // b


def _wrap16(lst):
    """dma_gather idx layout: [128, len//16] int16; idx i at [i%16, i//16],
    replicated across the 8 groups of 16 partitions."""
    n = len(lst)
    assert n % 16 == 0
    base = np.asarray(lst, dtype=np.int16).reshape(n // 16, 16).T  # [16, cols]
    return np.tile(base, (8, 1))  # [128, cols]


def build_plan(edge_src, edge_dst, n_nodes, n_cores):
    n_pad = _ceil_div(n_nodes, P * SUPER) * P * SUPER
    HALF = n_pad // 2
    B = _ceil_div(n_nodes, P)
    J = _ceil_div(B, n_cores)

    perm = np.argsort(edge_dst, kind="stable")
    dsts = edge_dst[perm].astype(np.int64)
    srcs = edge_src[perm].astype(np.int64)
    bounds = np.searchsorted(dsts, np.arange(B + 1) * P)

    # per-block A/B edge lists
    blkA, blkB = [], []
    for b in range(B):
        lo, hi = int(bounds[b]), int(bounds[b + 1])
        s, d = srcs[lo:hi], dsts[lo:hi]
        am = s < HALF
        blkA.append((s[am], d[am]))
        blkB.append((s[~am], d[~am]))

    chunksA = np.array([_ceil_div(len(blkA[b][0]), P) + 1 for b in range(B)])
    chunksB = np.array([_ceil_div(len(blkB[b][0]), P) + 1 for b in range(B)])
    order = np.argsort(-(chunksA + chunksB), kind="stable")

    CPBA, CPBB = [], []
    assign = -np.ones((n_cores, J), dtype=np.int64)
    for j in range(J):
        grp = order[j * n_cores : (j + 1) * n_cores]
        CPBA.append(int(chunksA[grp].max()))
        CPBB.append(int(chunksB[grp].max()))
        for c, b in enumerate(grp):
            assign[c, j] = b
    NCH = [a + b for a, b in zip(CPBA, CPBB)]
    TOTCH = int(sum(NCH))
    TA = int(sum(CPBA))
    TB = int(sum(CPBB))

    gA = np.zeros((n_cores, P, TA * 8), dtype=np.int16)
    gB = np.zeros((n_cores, P, TB * 8), dtype=np.int16)
    dcol = -np.ones((n_cores, P, TOTCH), dtype=np.float32)
    wab = np.zeros((n_cores, P, 2 * J), dtype=np.float32)

    for c in range(n_cores):
        cbA = cbB = cbN = 0
        for j in range(J):
            na, nb = CPBA[j], CPBB[j]
            b = assign[c, j]
            listA = np.zeros(na * P, dtype=np.int64)
            listB = np.zeros(nb * P, dtype=np.int64)
            if b >= 0:
                base = b * P
                sA, dA = blkA[b]
                sB, dB = blkB[b]
                listA[: len(sA)] = sA
                listB[: len(sB)] = sB - HALF
                inA = base < HALF
                hdr = np.arange(P) + (base - (0 if inA else HALF))
                if inA:
                    listA[(na - 1) * P :] = hdr
                    wab[c, :, 2 * j] = 1.0
                else:
                    listB[(nb - 1) * P :] = hdr
                    wab[c, :, 2 * j + 1] = 1.0
                # dcol for real edges (segment A then B), slot i -> [i%128, i//128]
                for lst_d, off in ((dA, 0), (dB, na)):
                    n = len(lst_d)
                    if n:
                        s_ = np.arange(n)
                        dcol[c, s_ & (P - 1), cbN + off + (s_ >> 7)] = (
                            lst_d - base
                        ).astype(np.float32)
            gA[c, :, cbA * 8 : (cbA + na) * 8] = _wrap16(listA)
            gB[c, :, cbB * 8 : (cbB + nb) * 8] = _wrap16(listB)
            cbA += na
            cbB += nb
            cbN += na + nb

    return {
        "n_pad": n_pad,
        "HALF": HALF,
        "B": B,
        "J": J,
        "CPBA": CPBA,
        "CPBB": CPBB,
        "NCH": NCH,
        "TOTCH": TOTCH,
        "TA": TA,
        "TB": TB,
        "CPBMAX": max(NCH),
        "assign": assign,
        "gA": gA,
        "gB": gB,
        "dcol": dcol,
        "wab": wab,
    }


def build_program(plan, n_cores, use_f32r=False):
    import concourse.bass as bass
    import concourse.tile as tile
    from concourse import bacc, mybir

    f32 = mybir.dt.float32
    i16 = mybir.dt.int16
    f32r = mybir.dt.float32r

    def mmcast(ap):
        return ap.bitcast(f32r) if use_f32r else ap

    n_pad = plan["n_pad"]
    HALF = plan["HALF"]
    J = plan["J"]
    CPBA, CPBB, NCH = plan["CPBA"], plan["CPBB"], plan["NCH"]
    TOTCH, TA, TB = plan["TOTCH"], plan["TA"], plan["TB"]
    cpbmax = plan["CPBMAX"]
    NT = n_pad // (P * SUPER)
    CG = IN_DIM // P

    nc = bacc.Bacc("TRN2", target_bir_lowering=False, debug=False,
                   num_devices=n_cores)

    hT = nc.dram_tensor("hT", [IN_DIM, n_pad], f32, kind="ExternalInput")
    waugT = nc.dram_tensor("waugT", [IN_DIM, RHS], f32, kind="ExternalInput")
    gA_d = nc.dram_tensor("gA", [P, TA * 8], i16, kind="ExternalInput")
    gB_d = nc.dram_tensor("gB", [P, TB * 8], i16, kind="ExternalInput")
    dcol_d = nc.dram_tensor("dcol", [P, TOTCH], f32, kind="ExternalInput")
    wab_d = nc.dram_tensor("wab", [P, 2 * J], f32, kind="ExternalInput")
    iota_d = nc.dram_tensor("iota", [P, P], f32, kind="ExternalInput")
    ident_d = nc.dram_tensor("ident", [P, P], f32, kind="ExternalInput")
    out_d = nc.dram_tensor("out", [J * P, OUT_DIM], f32, kind="ExternalOutput")
    whaug = nc.dram_tensor("whaug", [n_pad, ROW], f32)

    with tile.TileContext(nc) as tc, ExitStack() as ctx:
        consts = ctx.enter_context(tc.tile_pool(name="consts", bufs=1))
        ctx1 = ctx.enter_context(ExitStack())
        p1in = ctx1.enter_context(tc.tile_pool(name="p1in", bufs=3))
        p1ps = ctx1.enter_context(tc.tile_pool(name="p1ps", bufs=2, space="PSUM"))
        p1st = ctx1.enter_context(tc.tile_pool(name="p1st", bufs=3))

        waug_sb = consts.tile([P, CG, RHS], f32)
        nc.sync.dma_start(out=waug_sb[:],
                          in_=waugT.ap().rearrange("(g p) r -> p g r", p=P))
        iota_sb = consts.tile([P, P], f32)
        nc.sync.dma_start(out=iota_sb[:], in_=iota_d.ap())
        ident_sb = consts.tile([P, P], f32)
        nc.sync.dma_start(out=ident_sb[:], in_=ident_d.ap())
        gA_sb = consts.tile([P, TA * 8], i16)
        nc.sync.dma_start(out=gA_sb[:], in_=gA_d.ap())
        gB_sb = consts.tile([P, TB * 8], i16)
        nc.sync.dma_start(out=gB_sb[:], in_=gB_d.ap())
        dcol_sb = consts.tile([P, TOTCH], f32)
        nc.sync.dma_start(out=dcol_sb[:], in_=dcol_d.ap())
        wab_sb = consts.tile([P, 2 * J], f32)
        nc.sync.dma_start(out=wab_sb[:], in_=wab_d.ap())

        # ---- phase 1 ----
        hT_r = hT.ap().rearrange("(g p) n -> p g n", p=P)
        wh_r = whaug.ap().rearrange("(i t p) r -> i p t r", t=SUPER, p=P)
        for it in range(NT):
            ht = p1in.tile([P, CG, SUPER * P], f32)
            nc.sync.dma_start(
                out=ht[:], in_=hT_r[:, :, it * SUPER * P : (it + 1) * SUPER * P]
            )
            ps = p1ps.tile([P, SUPER, 512], f32)
            for t in range(SUPER):
                for g in range(CG):
                    nc.tensor.matmul(
                        out=ps[:, t, 0:RHS],
                        lhsT=mmcast(ht[:, g, t * P : (t + 1) * P]),
                        rhs=mmcast(waug_sb[:, g, :]),
                        start=(g == 0),
                        stop=(g == CG - 1),
                    )
            st = p1st.tile([P, SUPER, ROW], f32)
            nc.vector.memset(st[:, :, RHS:ROW], 0.0)
            nc.scalar.copy(out=st[:, :, 0:RHS], in_=ps[:, :, 0:RHS])
            nc.gpsimd.dma_start(out=wh_r[it], in_=st[:])

        ctx1.close()
        tc.strict_bb_all_engine_barrier()

        # ---- phase 2 ----
        m0p = ctx.enter_context(tc.tile_pool(name="m0p", bufs=2))
        selp = ctx.enter_context(tc.tile_pool(name="selp", bufs=2))
        sps = ctx.enter_context(tc.tile_pool(name="sps", bufs=3, space="PSUM"))
        ssb = ctx.enter_context(tc.tile_pool(name="ssb", bufs=3))
        adp = ctx.enter_context(tc.tile_pool(name="adp", bufs=2, space="PSUM"))
        accp = ctx.enter_context(tc.tile_pool(name="accp", bufs=2, space="PSUM"))
        scp = ctx.enter_context(tc.tile_pool(name="scp", bufs=2))
        outp = ctx.enter_context(tc.tile_pool(name="outp", bufs=2))
        smallp = ctx.enter_context(tc.tile_pool(name="smallp", bufs=4))

        tabA = whaug.ap()[0:HALF, :]
        tabB = whaug.ap()[HALF:n_pad, :]
        cbA = cbB = cbN = 0
        for j in range(J):
            na, nb, nch = CPBA[j], CPBB[j], NCH[j]
            m0t = m0p.tile([P, cpbmax, ROW], f32)
            GMAX = 8  # chunks per dma_gather call (<=1024 descriptors)
            for tab, nseg, cb, gsb, off in (
                (tabA, na, cbA, gA_sb, 0),
                (tabB, nb, cbB, gB_sb, na),
            ):
                for c0 in range(0, nseg, GMAX):
                    cn = min(GMAX, nseg - c0)
                    nc.gpsimd.dma_gather(
                        out_ap=m0t[:, off + c0 : off + c0 + cn, :],
                        in_ap=tab,
                        idxs_ap=gsb[:, (cb + c0) * 8 : (cb + c0 + cn) * 8],
                        num_idxs=cn * P,
                        num_idxs_reg=cn * P,
                        elem_size=ROW,
                        elem_step=ROW,
                    )
            # a_dst[128,8] = hdrA.aux*wA + hdrB.aux*wB
            ad_sb = smallp.tile([P, K], f32)
            t1 = smallp.tile([P, K], f32)
            nc.vector.tensor_scalar(
                out=t1[:], in0=m0t[:, na - 1, AUX : AUX + K],
                scalar1=wab_sb[:, 2 * j : 2 * j + 1], scalar2=None,
                op0=mybir.AluOpType.mult,
            )
            nc.vector.scalar_tensor_tensor(
                out=ad_sb[:], in0=m0t[:, nch - 1, AUX : AUX + K],
                scalar=wab_sb[:, 2 * j + 1 : 2 * j + 2],
                in1=t1[:], op0=mybir.AluOpType.mult, op1=mybir.AluOpType.add,
            )
            # batched one-hot masks
            sel = selp.tile([P, cpbmax, P], f32)
            iv = iota_sb[:]
            dview = dcol_sb[:, cbN : cbN + nch]
            nc.vector.tensor_tensor(
                out=sel[:, 0:nch, :],
                in0=bass.AP(tensor=iv.tensor, offset=iv.offset,
                            ap=[iv.ap[0], [0, nch], [1, P]]),
                in1=bass.AP(tensor=dview.tensor, offset=dview.offset,
                            ap=[dview.ap[0], [1, nch], [0, P]]),
                op=mybir.AluOpType.is_equal,
            )
            # per-chunk: S = sel_ci^T (PE), a_dst_e = S.T @ a_dst
            adst = adp.tile([P, cpbmax, K], f32)
            for ci in range(nch):
                s_ps = sps.tile([P, P], f32)
                nc.tensor.transpose(out=s_ps[:], in_=sel[:, ci, :],
                                    identity=ident_sb[:])
                s_sb = ssb.tile([P, P], f32)
                nc.scalar.copy(out=s_sb[:], in_=s_ps[:])
                nc.tensor.matmul(out=adst[:, ci, :], lhsT=s_sb[:], rhs=ad_sb[:],
                                 start=True, stop=True)
            # e_exp = exp(leaky(a_src + a_dst_e)) -> M0 aux
            aux = m0t[:, 0:nch, AUX : AUX + K]
            s_t = scp.tile([P, cpbmax, K], f32)
            nc.vector.tensor_tensor(out=s_t[:, 0:nch, :], in0=aux,
                                    in1=adst[:, 0:nch, :],
                                    op=mybir.AluOpType.add)
            lk = scp.tile([P, cpbmax, K], f32)
            nc.vector.scalar_tensor_tensor(
                out=lk[:, 0:nch, :], in0=s_t[:, 0:nch, :], scalar=NEG_SLOPE,
                in1=s_t[:, 0:nch, :],
                op0=mybir.AluOpType.mult, op1=mybir.AluOpType.max,
            )
            nc.scalar.activation(out=aux, in_=lk[:, 0:nch, :],
                                 func=mybir.ActivationFunctionType.Exp)
            # msgs *= bcast(e_exp)
            msg4 = m0t[:, 0:nch, 0:OUT_DIM].rearrange("p n (k d) -> p n k d", k=K)
            nc.vector.tensor_tensor(
                out=msg4, in0=msg4,
                in1=bass.AP(tensor=aux.tensor, offset=aux.offset,
                            ap=[aux.ap[0], [ROW, nch], [1, K], [0, DK]]),
                op=mybir.AluOpType.mult,
            )
            acc = accp.tile([P, RHS], f32)
            for ci in range(nch):
                nc.tensor.matmul(
                    out=acc[:],
                    lhsT=mmcast(sel[:, ci, :]),
                    rhs=mmcast(m0t[:, ci, 0:RHS]),
                    start=(ci == 0),
                    stop=(ci == nch - 1),
                )
            r = smallp.tile([P, K], f32)
            nc.vector.tensor_scalar(
                out=r[:], in0=acc[:, AUX : AUX + K], scalar1=1e-38, scalar2=None,
                op0=mybir.AluOpType.add,
            )
            nc.vector.reciprocal(out=r[:], in_=r[:])
            ot = outp.tile([P, OUT_DIM], f32)
            nc.vector.tensor_tensor(
                out=ot[:], in0=acc[:, 0:OUT_DIM],
                in1=r[:].to_broadcast([P, K, DK]),
                op=mybir.AluOpType.mult,
            )
            nc.sync.dma_start(out=out_d.ap()[j * P : (j + 1) * P, :], in_=ot[:])
            cbA += na
            cbB += nb
            cbN += nch

    nc.compile()
    return nc


def run(h, edge_src, edge_dst, W, attn, n_cores=N_CORES, trace=False,
        use_f32r=False):
    from concourse.bass_utils import run_bass_kernel_spmd

    n_nodes = h.shape[0]
    h = np.asarray(h, dtype=np.float32)
    W = np.asarray(W, dtype=np.float32)
    attn = np.asarray(attn, dtype=np.float32)
    edge_src = np.asarray(edge_src)
    edge_dst = np.asarray(edge_dst)

    plan = build_plan(edge_src, edge_dst, n_nodes, n_cores)
    n_pad = plan["n_pad"]
    hTd = np.zeros((IN_DIM, n_pad), dtype=np.float32)
    hTd[:, :n_nodes] = h.T
    c = (attn[:, :, None] * W.reshape(K, DK, IN_DIM)).sum(axis=1)
    waugT = np.concatenate([W.T, c.T], axis=1).astype(np.float32)
    iota = np.tile(np.arange(P, dtype=np.float32), (P, 1))
    ident = np.eye(P, dtype=np.float32)

    nc = build_program(plan, n_cores, use_f32r=use_f32r)

    in_maps = []
    for cix in range(n_cores):
        in_maps.append({
            "hT": hTd,
            "waugT": waugT,
            "gA": plan["gA"][cix],
            "gB": plan["gB"][cix],
            "dcol": plan["dcol"][cix],
            "wab": plan["wab"][cix],
            "iota": iota,
            "ident": ident,
        })
    try:
        res = run_bass_kernel_spmd(nc, in_maps, list(range(n_cores)), trace=trace)
    except Exception:
        if not trace:
            raise
        res = run_bass_kernel_spmd(nc, in_maps, list(range(n_cores)), trace=False)

    out_full = np.zeros((plan["B"] * P, OUT_DIM), dtype=np.float32)
    for cix in range(n_cores):
        o = res.results[cix]["out"]
        for j in range(plan["J"]):
            b = plan["assign"][cix, j]
            if b >= 0:
                out_full[b * P : (b + 1) * P] = o[j * P : (j + 1) * P]
    out = out_full[:n_nodes].reshape(n_nodes, K, DK)
    return out, res


def kernel(h, edge_src, edge_dst, W, attn):
    out, _ = run(h, edge_src, edge_dst, W, attn)
    return out



# revision 8
# speedup vs baseline: 1.9689x; 1.9689x over previous
"""NeighborRoutingConv (GAT-style multi-head edge-softmax message passing) on 8 trn2 cores.

Strategy (v5, bf16 edition):
  - Host folds attn into the weight matrix and PERMUTES Wh columns d-major
    (col = d*8+k) so the per-edge alpha broadcast has a packed last dim
    (DVE 2x mode).  One bf16 matmul per node tile emits
    whaug[n] = [ Wh-perm (256 bf16) ; e_slot (8 bf16, junk) ; a (8 f32) ; pad ]
    with 384-bf16 (768 B) row stride (dma_gather needs 256B multiples).
  - Phase 1 (replicated on every core): compute whaug for all N nodes into
    core-local DRAM.  hT/waugT are bf16 host inputs; PE runs 1 cycle/row.
  - Phase 2 (dst-sharded): edges grouped by 128-node destination blocks;
    blocks are grouped 8-per-slot with all 8 blocks of a slot in the SAME
    address half (src < HALF splits each block's edges into segment A/B for
    int16 gather indices; the dst block itself lives in exactly one half, so
    the single "header" chunk that gathers the block's own 128 dst rows has a
    compile-time position: end of segment A for A-slots, end of B for
    B-slots).  Per slot:
      * dma_gather whaug[src] rows (768 B) per segment -> M0 [128, nch, 384]
      * ad[128,8] = header aux (f32 -> fp16 copy)
      * sel[e, ci, d] = (dcol8[e,ci*8..]==iota[d]) one batched bf16 is_equal
        (dcol values pre-replicated x8 on host so the last AP dim is packed)
      * per chunk: PE-transpose sel_ci (bf16) -> psum; batched copy -> fp16;
        a_dst_e = selT.T @ ad  (fp16 matmul, 8 cols)
      * s = a_src(f32) + a_dst_e; e_exp = exp(leakyrelu(s)) -> bf16 e_slot
      * msgs *= bcast(e_exp)  (one batched DVE 2x multiply per slot)
      * per chunk one bf16 PE matmul accumulates
        [segment_sum(msgs) ; segment_sum(e_exp)] into PSUM [128, 264]
      * out_block = psum[:, :256] * bcast(1/(e_sum+eps)) -> bf16 DMA out.
  Softmax max-subtraction is skipped (|logit| <~ 26 so fp32/bf16 exp is safe).
  Host un-permutes output columns and upcasts to f32.
"""

import math
from contextlib import ExitStack

import numpy as np
import ml_dtypes

P = 128
IN_DIM = 256
OUT_DIM = 256
K = 8
DK = 32
ROW = 384          # whaug row stride in bf16 elements (768 B)
ECOL = 256         # e_exp slot: bf16 cols [256:264)
ACOL = 264         # a_src: bf16 cols [264:280) hold 8 f32 (byte 528..560)
STORE_COLS = 280   # phase-1 writes cols [0:280) (560 B rows)
RHS = 264          # matmul rhs width (msgs-perm 256 + e_exp 8)
NEG_SLOPE = 0.2
N_CORES = 8
SUPER = 4          # node tiles per phase-1 iteration (512 nodes)
TBATCH = 4         # sel transposes batched per psum tile / copy
GMAX = 8           # max chunks per dma_gather call (<=1024 descriptors)


def _ceil_div(a, b):
    return (a + b - 1) // b


def _wrap16(lst):
    """dma_gather idx layout: [128, len//16] int16; idx i at [i%16, i//16],
    replicated across the 8 groups of 16 partitions."""
    n = len(lst)
    assert n % 16 == 0
    base = np.asarray(lst, dtype=np.int16).reshape(n // 16, 16).T  # [16, cols]
    return np.tile(base, (8, 1))  # [128, cols]


def build_plan(edge_src, edge_dst, n_nodes, n_cores):
    n_pad = _ceil_div(n_nodes, P * SUPER) * P * SUPER
    HALF = n_pad // 2
    B = _ceil_div(n_nodes, P)
    BA = HALF // P  # blocks fully inside the A half: b in [0, BA)

    perm = np.argsort(edge_dst, kind="stable")
    dsts = edge_dst[perm].astype(np.int64)
    srcs = edge_src[perm].astype(np.int64)
    bounds = np.searchsorted(dsts, np.arange(B + 1) * P)

    blkA, blkB = [], []
    for b in range(B):
        lo, hi = int(bounds[b]), int(bounds[b + 1])
        s, d = srcs[lo:hi], dsts[lo:hi]
        am = s < HALF
        blkA.append((s[am], d[am]))
        blkB.append((s[~am], d[~am]))

    # raw per-block chunk counts; header (+1) goes to the block's own half
    rawA = np.array([_ceil_div(len(blkA[b][0]), P) for b in range(B)])
    rawB = np.array([_ceil_div(len(blkB[b][0]), P) for b in range(B)])
    chA = rawA + (np.arange(B) < BA)
    chB = rawB + (np.arange(B) >= BA)

    # group blocks 8-per-slot, same half per slot, big blocks first
    slots = []        # list of (np.array of block ids (or -1), is_A)
    for ids, is_A in ((np.arange(BA), True), (np.arange(BA, B), False)):
        tot = chA[ids] + chB[ids]
        order = ids[np.argsort(-tot, kind="stable")]
        for j0 in range(0, len(order), n_cores):
            grp = order[j0 : j0 + n_cores]
            if len(grp) < n_cores:
                grp = np.concatenate(
                    [grp, -np.ones(n_cores - len(grp), dtype=np.int64)]
                )
            slots.append((grp, is_A))
    J = len(slots)

    CPBA, CPBB, HPOS = [], [], []
    assign = -np.ones((n_cores, J), dtype=np.int64)
    for j, (grp, is_A) in enumerate(slots):
        real = grp[grp >= 0]
        na = int(chA[real].max()) if len(real) else 1
        nb = int(chB[real].max()) if len(real) else 1
        na = max(na, 1)
        nb = max(nb, 1)
        CPBA.append(na)
        CPBB.append(nb)
        HPOS.append(na - 1 if is_A else na + nb - 1)
        for c, b in enumerate(grp):
            assign[c, j] = b
    NCH = [a + b for a, b in zip(CPBA, CPBB)]
    TOTCH = int(sum(NCH))
    TA = int(sum(CPBA))
    TB = int(sum(CPBB))

    gA = np.zeros((n_cores, P, TA * 8), dtype=np.int16)
    gB = np.zeros((n_cores, P, TB * 8), dtype=np.int16)
    dcol8 = -np.ones((n_cores, P, TOTCH * 8), dtype=np.float32)

    for c in range(n_cores):
        cbA = cbB = cbN = 0
        for j, (grp, is_A) in enumerate(slots):
            na, nb = CPBA[j], CPBB[j]
            b = assign[c, j]
            listA = np.zeros(na * P, dtype=np.int64)
            listB = np.zeros(nb * P, dtype=np.int64)
            if b >= 0:
                base = b * P
                sA, dA = blkA[b]
                sB, dB = blkB[b]
                listA[: len(sA)] = sA
                listB[: len(sB)] = sB - HALF
                hdr = np.arange(P) + (base - (0 if is_A else HALF))
                if is_A:
                    listA[(na - 1) * P :] = hdr
                else:
                    listB[(nb - 1) * P :] = hdr
                for lst_d, off in ((dA, 0), (dB, na)):
                    n = len(lst_d)
                    if n:
                        s_ = np.arange(n)
                        v = (lst_d - base).astype(np.float32)
                        ci = cbN + off + (s_ >> 7)
                        pp = s_ & (P - 1)
                        for jj in range(8):
                            dcol8[c, pp, ci * 8 + jj] = v
            gA[c, :, cbA * 8 : (cbA + na) * 8] = _wrap16(listA)
            gB[c, :, cbB * 8 : (cbB + nb) * 8] = _wrap16(listB)
            cbA += na
            cbB += nb
            cbN += na + nb

    return {
        "n_pad": n_pad,
        "HALF": HALF,
        "B": B,
        "J": J,
        "CPBA": CPBA,
        "CPBB": CPBB,
        "NCH": NCH,
        "HPOS": HPOS,
        "TOTCH": TOTCH,
        "TA": TA,
        "TB": TB,
        "CPBMAX": max(NCH),
        "assign": assign,
        "gA": gA,
        "gB": gB,
        "dcol8": dcol8.astype(ml_dtypes.bfloat16),
    }


def build_program(plan, n_cores, use_f32r=False, ablate=()):
    ablate = set(ablate)
    import concourse.bass as bass
    import concourse.tile as tile
    from concourse import bacc, mybir

    f32 = mybir.dt.float32
    f16 = mybir.dt.float16
    bf16 = mybir.dt.bfloat16
    i16 = mybir.dt.int16

    n_pad = plan["n_pad"]
    HALF = plan["HALF"]
    J = plan["J"]
    CPBA, CPBB, NCH = plan["CPBA"], plan["CPBB"], plan["NCH"]
    HPOS = plan["HPOS"]
    TOTCH, TA, TB = plan["TOTCH"], plan["TA"], plan["TB"]
    cpbmax = plan["CPBMAX"]
    NT = n_pad // (P * SUPER)
    CG = IN_DIM // P

    nc = bacc.Bacc("TRN2", target_bir_lowering=False, debug=False,
                   num_devices=n_cores)

    hT = nc.dram_tensor("hT", [IN_DIM, n_pad], bf16, kind="ExternalInput")
    waugT = nc.dram_tensor("waugT", [IN_DIM, RHS], bf16, kind="ExternalInput")
    gA_d = nc.dram_tensor("gA", [P, TA * 8], i16, kind="ExternalInput")
    gB_d = nc.dram_tensor("gB", [P, TB * 8], i16, kind="ExternalInput")
    dcol8_d = nc.dram_tensor("dcol8", [P, TOTCH * 8], bf16, kind="ExternalInput")
    iota_d = nc.dram_tensor("iota", [P, P], bf16, kind="ExternalInput")
    ident_d = nc.dram_tensor("ident", [P, P], bf16, kind="ExternalInput")
    out_d = nc.dram_tensor("out", [J * P, OUT_DIM], bf16, kind="ExternalOutput")
    whaug = nc.dram_tensor("whaug", [n_pad, ROW], bf16)

    with tile.TileContext(nc) as tc, ExitStack() as ctx:
        consts = ctx.enter_context(tc.tile_pool(name="consts", bufs=1))
        ctx1 = ctx.enter_context(ExitStack())
        p1in = ctx1.enter_context(tc.tile_pool(name="p1in", bufs=3))
        p1ps = ctx1.enter_context(tc.tile_pool(name="p1ps", bufs=2, space="PSUM"))
        p1st = ctx1.enter_context(tc.tile_pool(name="p1st", bufs=3))

        waug_sb = consts.tile([P, CG, RHS], bf16)
        nc.sync.dma_start(out=waug_sb[:],
                          in_=waugT.ap().rearrange("(g p) r -> p g r", p=P))
        iota_sb = consts.tile([P, P], bf16)
        nc.sync.dma_start(out=iota_sb[:], in_=iota_d.ap())
        ident_sb = consts.tile([P, P], bf16)
        nc.sync.dma_start(out=ident_sb[:], in_=ident_d.ap())
        gA_sb = consts.tile([P, TA * 8], i16)
        nc.sync.dma_start(out=gA_sb[:], in_=gA_d.ap())
        gB_sb = consts.tile([P, TB * 8], i16)
        nc.sync.dma_start(out=gB_sb[:], in_=gB_d.ap())
        dcol8_sb = consts.tile([P, TOTCH * 8], bf16)
        nc.sync.dma_start(out=dcol8_sb[:], in_=dcol8_d.ap())

        # ---- phase 1 ----
        hT_r = hT.ap().rearrange("(g p) n -> p g n", p=P)
        wh_r = whaug.ap().rearrange("(i t p) r -> i p t r", t=SUPER, p=P)
        for it in range(NT if "phase1" not in ablate else 1):
            ht = p1in.tile([P, CG, SUPER * P], bf16)
            nc.sync.dma_start(
                out=ht[:], in_=hT_r[:, :, it * SUPER * P : (it + 1) * SUPER * P]
            )
            ps = p1ps.tile([P, SUPER, 512], f32)
            for t in range(SUPER):
                for g in range(CG):
                    nc.tensor.matmul(
                        out=ps[:, t, 0:RHS],
                        lhsT=ht[:, g, t * P : (t + 1) * P],
                        rhs=waug_sb[:, g, :],
                        start=(g == 0),
                        stop=(g == CG - 1),
                    )
            st = p1st.tile([P, SUPER, ROW], bf16)
            # Wh (perm) -> bf16 cols [0:256)
            nc.scalar.copy(out=st[:, :, 0:ECOL], in_=ps[:, :, 0:ECOL])
            # a -> f32 at bf16 cols [264:280)
            nc.vector.tensor_copy(
                out=st[:, :, ACOL : ACOL + 16].bitcast(f32),
                in_=ps[:, :, ECOL:RHS],
            )
            nc.sync.dma_start(out=wh_r[it][:, :, 0:STORE_COLS],
                              in_=st[:, :, 0:STORE_COLS])

        ctx1.close()
        tc.strict_bb_all_engine_barrier()

        if "phase2" in ablate:
            nc.compile()
            return nc

        # ---- phase 2 ----
        m0p = ctx.enter_context(tc.tile_pool(name="m0p", bufs=2))
        selp = ctx.enter_context(tc.tile_pool(name="selp", bufs=2))
        sps = ctx.enter_context(tc.tile_pool(name="sps", bufs=2, space="PSUM"))
        ssb = ctx.enter_context(tc.tile_pool(name="ssb", bufs=2))
        adp = ctx.enter_context(tc.tile_pool(name="adp", bufs=2, space="PSUM"))
        accp = ctx.enter_context(tc.tile_pool(name="accp", bufs=2, space="PSUM"))
        scp = ctx.enter_context(tc.tile_pool(name="scp", bufs=2))
        outp = ctx.enter_context(tc.tile_pool(name="outp", bufs=2))
        smallp = ctx.enter_context(tc.tile_pool(name="smallp", bufs=4))

        tabA = whaug.ap()[0:HALF, :]
        tabB = whaug.ap()[HALF:n_pad, :]
        cbA = cbB = cbN = 0
        for j in range(J):
            na, nb, nch, hpos = CPBA[j], CPBB[j], NCH[j], HPOS[j]
            m0t = m0p.tile([P, cpbmax, ROW], bf16)
            for tab, nseg, cb, gsb, off in (
                (tabA, na, cbA, gA_sb, 0),
                (tabB, nb, cbB, gB_sb, na),
            ):
                for c0 in range(0, nseg, GMAX):
                    cn = min(GMAX, nseg - c0)
                    nc.gpsimd.dma_gather(
                        out_ap=m0t[:, off + c0 : off + c0 + cn, :],
                        in_ap=tab,
                        idxs_ap=gsb[:, (cb + c0) * 8 : (cb + c0 + cn) * 8],
                        num_idxs=cn * P,
                        num_idxs_reg=cn * P,
                        elem_size=ROW,
                        elem_step=ROW,
                    )
            # a_dst of the block's own 128 rows, from the header chunk
            ad_sb = smallp.tile([P, K], f16)
            nc.vector.tensor_copy(
                out=ad_sb[:], in_=m0t[:, hpos, ACOL : ACOL + 16].bitcast(f32)
            )
            # batched one-hot masks: sel[e, ci, d] = (dcol[e,ci] == d)
            sel = selp.tile([P, cpbmax, P], bf16)
            iv = iota_sb[:]
            dview = dcol8_sb[:, cbN * 8 : (cbN + nch) * 8]
            nc.vector.tensor_tensor(
                out=sel[:, 0:nch, :].rearrange("p n (g k) -> p n g k", k=8),
                in0=bass.AP(tensor=iv.tensor, offset=iv.offset,
                            ap=[iv.ap[0], [0, nch], [8, 16], [1, 8]]),
                in1=bass.AP(tensor=dview.tensor, offset=dview.offset,
                            ap=[dview.ap[0], [8, nch], [0, 16], [1, 8]]),
                op=mybir.AluOpType.is_equal,
            )
            # per-chunk: S = sel_ci^T (PE, bf16); a_dst_e = S.T @ ad (fp16)
            adst = adp.tile([P, cpbmax, K], f32)
            if "adst" in ablate:
                nc.vector.memset(adst[:, 0:nch, :], 0.0)
            for c0 in range(0, nch, TBATCH) if "adst" not in ablate else ():
                cn = min(TBATCH, nch - c0)
                s_ps = sps.tile([P, TBATCH, P], bf16)
                for i in range(cn):
                    nc.tensor.transpose(out=s_ps[:, i, :],
                                        in_=sel[:, c0 + i, :],
                                        identity=ident_sb[:])
                s_sb = ssb.tile([P, TBATCH, P], f16)
                nc.scalar.copy(out=s_sb[:, 0:cn, :], in_=s_ps[:, 0:cn, :])
                for i in range(cn):
                    nc.tensor.matmul(out=adst[:, c0 + i, :],
                                     lhsT=s_sb[:, i, :], rhs=ad_sb[:],
                                     start=True, stop=True)
            # e_exp = exp(leaky(a_src + a_dst_e)) -> bf16 e_slot cols
            a_src = m0t[:, 0:nch, ACOL : ACOL + 16].bitcast(f32)
            s_t = scp.tile([P, cpbmax, K], f32)
            nc.vector.tensor_tensor(out=s_t[:, 0:nch, :], in0=a_src,
                                    in1=adst[:, 0:nch, :],
                                    op=mybir.AluOpType.add)
            lk = scp.tile([P, cpbmax, K], f32)
            nc.vector.scalar_tensor_tensor(
                out=lk[:, 0:nch, :], in0=s_t[:, 0:nch, :], scalar=NEG_SLOPE,
                in1=s_t[:, 0:nch, :],
                op0=mybir.AluOpType.mult, op1=mybir.AluOpType.max,
            )
            aux = m0t[:, 0:nch, ECOL : ECOL + K]
            nc.scalar.activation(out=aux, in_=lk[:, 0:nch, :],
                                 func=mybir.ActivationFunctionType.Exp)
            # msgs *= bcast(e_exp): one batched DVE 2x multiply (d-major perm)
            msg4 = m0t[:, 0:nch, 0:ECOL].rearrange("p n (d k) -> p n d k", k=8)
            nc.vector.tensor_tensor(
                out=msg4, in0=msg4,
                in1=bass.AP(tensor=aux.tensor, offset=aux.offset,
                            ap=[aux.ap[0], [ROW, nch], [0, DK], [1, K]]),
                op=mybir.AluOpType.mult,
            )
            acc = accp.tile([P, RHS], f32)
            for ci in range(nch):
                nc.tensor.matmul(
                    out=acc[:],
                    lhsT=sel[:, ci, :],
                    rhs=m0t[:, ci, 0:RHS],
                    start=(ci == 0),
                    stop=(ci == nch - 1),
                )
            r = smallp.tile([P, K], f32)
            nc.vector.tensor_scalar(
                out=r[:], in0=acc[:, ECOL:RHS], scalar1=1e-38, scalar2=None,
                op0=mybir.AluOpType.add,
            )
            nc.vector.reciprocal(out=r[:], in_=r[:])
            ot = outp.tile([P, OUT_DIM], bf16)
            rv = r[:]
            nc.vector.tensor_tensor(
                out=ot[:].rearrange("p (d k) -> p d k", k=8),
                in0=acc[:, 0:ECOL].rearrange("p (d k) -> p d k", k=8),
                in1=bass.AP(tensor=rv.tensor, offset=rv.offset,
                            ap=[rv.ap[0], [0, DK], [1, K]]),
                op=mybir.AluOpType.mult,
            )
            nc.sync.dma_start(out=out_d.ap()[j * P : (j + 1) * P, :], in_=ot[:])
            cbA += na
            cbB += nb
            cbN += nch

    nc.compile()
    return nc


def run(h, edge_src, edge_dst, W, attn, n_cores=N_CORES, trace=False,
        use_f32r=False):
    from concourse.bass_utils import run_bass_kernel_spmd

    n_nodes = h.shape[0]
    h = np.asarray(h, dtype=np.float32)
    W = np.asarray(W, dtype=np.float32)
    attn = np.asarray(attn, dtype=np.float32)
    edge_src = np.asarray(edge_src)
    edge_dst = np.asarray(edge_dst)

    plan = build_plan(edge_src, edge_dst, n_nodes, n_cores)
    n_pad = plan["n_pad"]
    hTd = np.zeros((IN_DIM, n_pad), dtype=np.float32)
    hTd[:, :n_nodes] = h.T
    # W rows permuted d-major: row (d*8+k) = W[k*32+d]
    Wperm = W.reshape(K, DK, IN_DIM).transpose(1, 0, 2).reshape(OUT_DIM, IN_DIM)
    c = (attn[:, :, None] * W.reshape(K, DK, IN_DIM)).sum(axis=1)
    waugT = np.concatenate([Wperm.T, c.T], axis=1).astype(np.float32)
    iota = np.tile(np.arange(P, dtype=np.float32), (P, 1))
    ident = np.eye(P, dtype=np.float32)

    nc = build_program(plan, n_cores, use_f32r=use_f32r)

    in_maps = []
    for cix in range(n_cores):
        in_maps.append({
            "hT": hTd.astype(ml_dtypes.bfloat16),
            "waugT": waugT.astype(ml_dtypes.bfloat16),
            "gA": plan["gA"][cix],
            "gB": plan["gB"][cix],
            "dcol8": plan["dcol8"][cix],
            "iota": iota.astype(ml_dtypes.bfloat16),
            "ident": ident.astype(ml_dtypes.bfloat16),
        })
    try:
        res = run_bass_kernel_spmd(nc, in_maps, list(range(n_cores)), trace=trace)
    except Exception:
        if not trace:
            raise
        res = run_bass_kernel_spmd(nc, in_maps, list(range(n_cores)), trace=False)

    out_full = np.zeros((plan["B"] * P, OUT_DIM), dtype=np.float32)
    for cix in range(n_cores):
        o = np.asarray(res.results[cix]["out"]).astype(np.float32)
        for j in range(plan["J"]):
            b = plan["assign"][cix, j]
            if b >= 0:
                out_full[b * P : (b + 1) * P] = o[j * P : (j + 1) * P]
    # un-permute columns: stored col = d*8+k -> [K, DK]
    out = out_full[:n_nodes].reshape(n_nodes, DK, K).transpose(0, 2, 1)
    return np.ascontiguousarray(out), res


def kernel(h, edge_src, edge_dst, W, attn):
    out, _ = run(h, edge_src, edge_dst, W, attn)
    return out


# revision 22
# speedup vs baseline: 2.9072x; 1.4766x over previous
"""NeighborRoutingConv (GAT-style multi-head edge-softmax message passing) on 8 trn2 cores.

Strategy (v6, all-gather edition):
  - Host folds attn into the weight matrix and PERMUTES Wh columns d-major
    (col = d*8+k) so the per-edge alpha broadcast has a packed last dim
    (DVE 2x mode).  One bf16 matmul per node tile emits
    whaug[n] = [ Wh-perm (256 bf16) ; e_slot (8 bf16) ; a (8 f32) ; pad ]
    declared as f32[192] rows (768 B) plus a compact a-table atab f32[64]
    (256 B rows, first 8 = a) for per-edge destination lookups.
  - Phase 1 (replicated on every core): compute whaug + atab for all N nodes
    into core-local DRAM.  DMAs are spread across the SP (loads), Pool
    (whaug stores) and DVE (atab stores) queues.
  - Phase 2 (dst-sharded): edges grouped by 128-node destination blocks;
    blocks are grouped 8-per-slot with all 8 blocks of a slot in the SAME
    address half (int16 gather indices; src splits each block's edge list
    into segment A/B).  Everything per-edge is fetched by dma_gather:
      * whaug[src] rows -> M0 [128, nch, 192] f32 (Wh + a_src in-row)
      * one-hot sel rows from a 256-row identity table (idx = in-block dst,
        128 -> zero row for padding)  -> bf16 [128, nch, 128] via bitcast
      * a_dst rows from atab[dst] (slot's half known at compile time)
    Then per slot: s = a_src + a_dst; e_exp = exp(leakyrelu(s)) -> e_slot;
    msgs *= bcast(e_exp) (batched DVE 2x); per chunk one bf16 PE matmul
    accumulates [segment_sum(msgs) ; segment_sum(e_exp)] into PSUM [128,264];
    out_block = psum[:, :256] / bcast(e_sum+eps) -> bf16 DMA out.
    The per-slot tail is split at the A|B segment boundary so the A-half
    work overlaps the B-segment gathers.
  Softmax max-subtraction is skipped (|logit| <~ 26 so fp32/bf16 exp is safe).
  Host un-permutes output columns and upcasts to f32.
"""

import math
from contextlib import ExitStack

import numpy as np
import ml_dtypes

P = 128
IN_DIM = 256
OUT_DIM = 256
K = 8
DK = 32
ROWF = 192         # whaug row stride in f32 elements (768 B)
ROWB = 384         # same row in bf16 units
ECOLF = 128        # e_exp slot: f32 cols [128:132) == bf16 cols [256:264)
ACOLF = 132        # a_src: f32 cols [132:140)
STORE_COLS = 140   # phase-1 writes f32 cols [0:140) (560 B rows)
ATROW = 64         # atab row stride in f32 (256 B)
RHS = 264          # matmul rhs width in bf16 (msgs-perm 256 + e_exp 8)
NEG_SLOPE = 0.2
N_CORES = 8
SUPER = 4          # node tiles per phase-1 iteration (512 nodes)
GMAX = 8           # max chunks per dma_gather call (<=1024 descriptors)


def _ceil_div(a, b):
    return (a + b - 1) // b


def _wrap16(lst):
    """dma_gather idx layout: [128, len//16] int16; idx i at [i%16, i//16],
    replicated across the 8 groups of 16 partitions."""
    n = len(lst)
    assert n % 16 == 0
    base = np.asarray(lst, dtype=np.int16).reshape(n // 16, 16).T  # [16, cols]
    return np.tile(base, (8, 1))  # [128, cols]


def build_plan(edge_src, edge_dst, n_nodes, n_cores):
    n_pad = _ceil_div(n_nodes, P * SUPER) * P * SUPER
    HALF = n_pad // 2
    B = _ceil_div(n_nodes, P)
    BA = HALF // P  # blocks fully inside the A half: b in [0, BA)

    perm = np.argsort(edge_dst, kind="stable")
    dsts = edge_dst[perm].astype(np.int64)
    srcs = edge_src[perm].astype(np.int64)
    bounds = np.searchsorted(dsts, np.arange(B + 1) * P)

    blkA, blkB = [], []
    for b in range(B):
        lo, hi = int(bounds[b]), int(bounds[b + 1])
        s, d = srcs[lo:hi], dsts[lo:hi]
        am = s < HALF
        blkA.append((s[am], d[am]))
        blkB.append((s[~am], d[~am]))

    chA = np.array([_ceil_div(len(blkA[b][0]), P) for b in range(B)])
    chB = np.array([_ceil_div(len(blkB[b][0]), P) for b in range(B)])

    # group blocks 8-per-slot, same half per slot, big blocks first
    slots = []  # (np.array of block ids (or -1), is_A)
    for ids, is_A in ((np.arange(BA), True), (np.arange(BA, B), False)):
        order = ids[np.argsort(-(chA[ids] * 1000 + chB[ids]), kind="stable")]
        for j0 in range(0, len(order), n_cores):
            grp = order[j0 : j0 + n_cores]
            if len(grp) < n_cores:
                grp = np.concatenate(
                    [grp, -np.ones(n_cores - len(grp), dtype=np.int64)]
                )
            slots.append((grp, is_A))
    J = len(slots)

    CPBA, CPBB, ISA = [], [], []
    assign = -np.ones((n_cores, J), dtype=np.int64)
    for j, (grp, is_A) in enumerate(slots):
        real = grp[grp >= 0]
        na = max(int(chA[real].max()) if len(real) else 1, 1)
        nb = max(int(chB[real].max()) if len(real) else 1, 1)
        CPBA.append(na)
        CPBB.append(nb)
        ISA.append(is_A)
        for c, b in enumerate(grp):
            assign[c, j] = b
    NCH = [a + b for a, b in zip(CPBA, CPBB)]
    TOTCH = int(sum(NCH))
    TA = int(sum(CPBA))
    TB = int(sum(CPBB))

    gA = np.zeros((n_cores, P, TA * 8), dtype=np.int16)
    gB = np.zeros((n_cores, P, TB * 8), dtype=np.int16)
    gS = np.full((n_cores, P, TOTCH * 8), 128, dtype=np.int16)
    gD = np.zeros((n_cores, P, TOTCH * 8), dtype=np.int16)

    for c in range(n_cores):
        cbA = cbB = cbN = 0
        for j, (grp, is_A) in enumerate(slots):
            na, nb = CPBA[j], CPBB[j]
            b = assign[c, j]
            listA = np.zeros(na * P, dtype=np.int64)
            listB = np.zeros(nb * P, dtype=np.int64)
            listS = np.full((na + nb) * P, 128, dtype=np.int64)
            listD = np.zeros((na + nb) * P, dtype=np.int64)
            if b >= 0:
                base = b * P
                hb = 0 if is_A else HALF
                sA, dA = blkA[b]
                sB, dB = blkB[b]
                listA[: len(sA)] = sA
                listB[: len(sB)] = sB - HALF
                listS[: len(sA)] = dA - base
                listS[na * P : na * P + len(sB)] = dB - base
                listD[: len(sA)] = dA - hb
                listD[na * P : na * P + len(sB)] = dB - hb
            gA[c, :, cbA * 8 : (cbA + na) * 8] = _wrap16(listA)
            gB[c, :, cbB * 8 : (cbB + nb) * 8] = _wrap16(listB)
            gS[c, :, cbN * 8 : (cbN + na + nb) * 8] = _wrap16(listS)
            gD[c, :, cbN * 8 : (cbN + na + nb) * 8] = _wrap16(listD)
            cbA += na
            cbB += nb
            cbN += na + nb

    return {
        "n_pad": n_pad,
        "HALF": HALF,
        "B": B,
        "J": J,
        "CPBA": CPBA,
        "CPBB": CPBB,
        "NCH": NCH,
        "ISA": ISA,
        "TOTCH": TOTCH,
        "TA": TA,
        "TB": TB,
        "CPBMAX": max(NCH),
        "assign": assign,
        "gA": gA,
        "gB": gB,
        "gS": gS,
        "gD": gD,
    }


def build_program(plan, n_cores, use_f32r=False, ablate=()):
    ablate = set(ablate)
    import concourse.bass as bass
    import concourse.tile as tile
    from concourse import bacc, mybir

    def bass_AP(base, offset, ap):
        return bass.AP(tensor=base.tensor, offset=offset, ap=ap)

    f32 = mybir.dt.float32
    bf16 = mybir.dt.bfloat16
    i16 = mybir.dt.int16

    n_pad = plan["n_pad"]
    HALF = plan["HALF"]
    J = plan["J"]
    CPBA, CPBB, NCH = plan["CPBA"], plan["CPBB"], plan["NCH"]
    ISA = plan["ISA"]
    TOTCH, TA, TB = plan["TOTCH"], plan["TA"], plan["TB"]
    cpbmax = plan["CPBMAX"]
    NT = n_pad // (P * SUPER)
    CG = IN_DIM // P

    nc = bacc.Bacc("TRN2", target_bir_lowering=False, debug=False,
                   num_devices=n_cores)

    hT = nc.dram_tensor("hT", [IN_DIM, n_pad], bf16, kind="ExternalInput")
    waugT = nc.dram_tensor("waugT", [IN_DIM, RHS], bf16, kind="ExternalInput")
    gA_d = nc.dram_tensor("gA", [P, TA * 8], i16, kind="ExternalInput")
    gB_d = nc.dram_tensor("gB", [P, TB * 8], i16, kind="ExternalInput")
    gS_d = nc.dram_tensor("gS", [P, TOTCH * 8], i16, kind="ExternalInput")
    gD_d = nc.dram_tensor("gD", [P, TOTCH * 8], i16, kind="ExternalInput")
    onehot_d = nc.dram_tensor("onehot", [256, P], bf16, kind="ExternalInput")
    out_d = nc.dram_tensor("out", [J * P, OUT_DIM], bf16, kind="ExternalOutput")
    # +4 pad rows: the a_dst gather reads a 64-f32 window starting at col 132,
    # which runs 4 f32 past the row end for the last table row.
    whaug = nc.dram_tensor("whaug", [n_pad + 4, ROWF], f32)

    with tile.TileContext(nc) as tc, ExitStack() as ctx:
        consts = ctx.enter_context(tc.tile_pool(name="consts", bufs=1))
        ctx1 = ctx.enter_context(ExitStack())
        p1in = ctx1.enter_context(tc.tile_pool(name="p1in", bufs=3))
        p1ps = ctx1.enter_context(tc.tile_pool(name="p1ps", bufs=2, space="PSUM"))
        p1st = ctx1.enter_context(tc.tile_pool(name="p1st", bufs=3))

        waug_sb = consts.tile([P, CG, RHS], bf16)
        nc.sync.dma_start(out=waug_sb[:],
                          in_=waugT.ap().rearrange("(g p) r -> p g r", p=P))
        gA_sb = consts.tile([P, TA * 8], i16)
        nc.sync.dma_start(out=gA_sb[:], in_=gA_d.ap())
        gB_sb = consts.tile([P, TB * 8], i16)
        nc.sync.dma_start(out=gB_sb[:], in_=gB_d.ap())
        gS_sb = consts.tile([P, TOTCH * 8], i16)
        nc.sync.dma_start(out=gS_sb[:], in_=gS_d.ap())
        gD_sb = consts.tile([P, TOTCH * 8], i16)
        nc.sync.dma_start(out=gD_sb[:], in_=gD_d.ap())

        # ---- phase 1 ----
        hT_r = hT.ap().rearrange("(g p) n -> p g n", p=P)
        wh_r = whaug.ap()[0:n_pad, :].rearrange("(i t p) r -> i p t r",
                                                t=SUPER, p=P)
        for it in range(NT if "phase1" not in ablate else 1):
            ht = p1in.tile([P, CG, SUPER * P], bf16)
            nc.sync.dma_start(
                out=ht[:], in_=hT_r[:, :, it * SUPER * P : (it + 1) * SUPER * P]
            )
            ps = p1ps.tile([P, SUPER, 512], f32)
            for t in range(SUPER):
                for g in range(CG):
                    nc.tensor.matmul(
                        out=ps[:, t, 0:RHS],
                        lhsT=ht[:, g, t * P : (t + 1) * P],
                        rhs=waug_sb[:, g, :],
                        start=(g == 0),
                        stop=(g == CG - 1),
                    )
            st = p1st.tile([P, SUPER, ROWF], f32)
            # Wh (perm) -> bf16 cols [0:256); split Act/DVE to balance queues
            nc.scalar.copy(out=st[:, 0:3, 0:ECOLF].bitcast(bf16),
                           in_=ps[:, 0:3, 0:OUT_DIM])
            nc.vector.tensor_copy(out=st[:, 3:SUPER, 0:ECOLF].bitcast(bf16),
                                  in_=ps[:, 3:SUPER, 0:OUT_DIM])
            # a -> f32 cols [132:140)
            nc.vector.tensor_copy(out=st[:, :, ACOLF : ACOLF + K],
                                  in_=ps[:, :, OUT_DIM:RHS])
            nc.gpsimd.dma_start(out=wh_r[it][:, :, 0:STORE_COLS],
                                in_=st[:, :, 0:STORE_COLS])

        ctx1.close()
        tc.strict_bb_all_engine_barrier()

        if "phase2" in ablate:
            nc.compile()
            return nc

        # ---- phase 2 ----
        m0p = ctx.enter_context(tc.tile_pool(name="m0p", bufs=3))
        selp = ctx.enter_context(tc.tile_pool(name="selp", bufs=3))
        adfp = ctx.enter_context(tc.tile_pool(name="adfp", bufs=3))
        accp = ctx.enter_context(tc.tile_pool(name="accp", bufs=3, space="PSUM"))
        scp = ctx.enter_context(tc.tile_pool(name="scp", bufs=2))
        outp = ctx.enter_context(tc.tile_pool(name="outp", bufs=2))
        smallp = ctx.enter_context(tc.tile_pool(name="smallp", bufs=4))

        tabA = whaug.ap()[0:HALF, :]
        tabB = whaug.ap()[HALF:n_pad, :]
        # a_dst windows: 64-f32 reads starting at col 132 of each row (runs
        # into the next row's head / the pad rows; only [0:8) is used)
        wt = whaug.ap()
        atA = bass_AP(wt, ACOLF, [[ROWF, HALF], [1, ATROW]])
        atB = bass_AP(wt, HALF * ROWF + ACOLF, [[ROWF, HALF], [1, ATROW]])
        oh = onehot_d.ap().bitcast(f32)  # [256, 64] f32
        cbA = cbB = cbN = 0
        for j in range(J):
            na, nb, nch, is_A = CPBA[j], CPBB[j], NCH[j], ISA[j]
            m0t = m0p.tile([P, cpbmax, ROWF], f32)
            for tab, nseg, cb, gsb, off in (
                (tabA, na, cbA, gA_sb, 0),
                (tabB, nb, cbB, gB_sb, na),
            ):
                for c0 in range(0, nseg, GMAX):
                    cn = min(GMAX, nseg - c0)
                    nc.gpsimd.dma_gather(
                        out_ap=m0t[:, off + c0 : off + c0 + cn, :],
                        in_ap=tab,
                        idxs_ap=gsb[:, (cb + c0) * 8 : (cb + c0 + cn) * 8],
                        num_idxs=cn * P,
                        num_idxs_reg=cn * P,
                        elem_size=ROWF,
                        elem_step=ROWF,
                    )
            # one-hot sel rows (bf16 payload in a f32-declared gather)
            selg = selp.tile([P, cpbmax, ATROW], f32)
            adf = adfp.tile([P, cpbmax, ATROW], f32)
            at = atA if is_A else atB
            for dst_t, srct, gsb2, estep in (
                (selg, oh, gS_sb, ATROW),
                (adf, at, gD_sb, ROWF),
            ):
                for c0 in range(0, nch, GMAX):
                    cn = min(GMAX, nch - c0)
                    nc.gpsimd.dma_gather(
                        out_ap=dst_t[:, c0 : c0 + cn, :],
                        in_ap=srct,
                        idxs_ap=gsb2[:, (cbN + c0) * 8 : (cbN + c0 + cn) * 8],
                        num_idxs=cn * P,
                        num_idxs_reg=cn * P,
                        elem_size=ATROW,
                        elem_step=estep,
                    )
            # tail, split at the A|B boundary to overlap B gathers
            s_t = scp.tile([P, cpbmax, K], f32)
            lk = scp.tile([P, cpbmax, K], f32)
            acc = accp.tile([P, RHS], f32)
            for lo, hi in ((0, na), (na, nch)):
                if hi <= lo:
                    continue
                n_r = hi - lo
                nc.vector.tensor_tensor(
                    out=s_t[:, lo:hi, :],
                    in0=m0t[:, lo:hi, ACOLF : ACOLF + K],
                    in1=adf[:, lo:hi, 0:K],
                    op=mybir.AluOpType.add,
                )
                nc.vector.scalar_tensor_tensor(
                    out=lk[:, lo:hi, :], in0=s_t[:, lo:hi, :], scalar=NEG_SLOPE,
                    in1=s_t[:, lo:hi, :],
                    op0=mybir.AluOpType.mult, op1=mybir.AluOpType.max,
                )
                aux = m0t[:, lo:hi, ECOLF : ECOLF + 4].bitcast(bf16)  # [P,n_r,8]
                nc.scalar.activation(out=aux, in_=lk[:, lo:hi, :],
                                     func=mybir.ActivationFunctionType.Exp)
                msg4 = (m0t[:, lo:hi, 0:ECOLF].bitcast(bf16)
                        .rearrange("p n (d k) -> p n d k", k=8))
                nc.vector.tensor_tensor(
                    out=msg4, in0=msg4,
                    in1=bass.AP(tensor=aux.tensor, offset=aux.offset,
                                ap=[aux.ap[0], [ROWB, n_r], [0, DK], [1, K]]),
                    op=mybir.AluOpType.mult,
                )
                for ci in range(lo, hi):
                    nc.tensor.matmul(
                        out=acc[:],
                        lhsT=selg[:, ci, :].bitcast(bf16),
                        rhs=m0t[:, ci, 0 : RHS // 2].bitcast(bf16),
                        start=(ci == 0),
                        stop=(ci == nch - 1),
                    )
            r = smallp.tile([P, K], f32)
            nc.vector.tensor_scalar(
                out=r[:], in0=acc[:, OUT_DIM:RHS], scalar1=1e-38, scalar2=None,
                op0=mybir.AluOpType.add,
            )
            nc.vector.reciprocal(out=r[:], in_=r[:])
            ot = outp.tile([P, OUT_DIM], bf16)
            rv = r[:]
            nc.vector.tensor_tensor(
                out=ot[:].rearrange("p (d k) -> p d k", k=8),
                in0=acc[:, 0:OUT_DIM].rearrange("p (d k) -> p d k", k=8),
                in1=bass.AP(tensor=rv.tensor, offset=rv.offset,
                            ap=[rv.ap[0], [0, DK], [1, K]]),
                op=mybir.AluOpType.mult,
            )
            nc.sync.dma_start(out=out_d.ap()[j * P : (j + 1) * P, :], in_=ot[:])
            cbA += na
            cbB += nb
            cbN += nch

    nc.compile()
    return nc


def run(h, edge_src, edge_dst, W, attn, n_cores=N_CORES, trace=False,
        use_f32r=False):
    from concourse.bass_utils import run_bass_kernel_spmd

    n_nodes = h.shape[0]
    h = np.asarray(h, dtype=np.float32)
    W = np.asarray(W, dtype=np.float32)
    attn = np.asarray(attn, dtype=np.float32)
    edge_src = np.asarray(edge_src)
    edge_dst = np.asarray(edge_dst)

    plan = build_plan(edge_src, edge_dst, n_nodes, n_cores)
    n_pad = plan["n_pad"]
    hTd = np.zeros((IN_DIM, n_pad), dtype=np.float32)
    hTd[:, :n_nodes] = h.T
    # W rows permuted d-major: row (d*8+k) = W[k*32+d]
    Wperm = W.reshape(K, DK, IN_DIM).transpose(1, 0, 2).reshape(OUT_DIM, IN_DIM)
    c = (attn[:, :, None] * W.reshape(K, DK, IN_DIM)).sum(axis=1)
    waugT = np.concatenate([Wperm.T, c.T], axis=1).astype(np.float32)
    onehot = np.zeros((256, P), dtype=ml_dtypes.bfloat16)
    onehot[:P] = np.eye(P, dtype=np.float32).astype(ml_dtypes.bfloat16)

    nc = build_program(plan, n_cores, use_f32r=use_f32r)

    in_maps = []
    for cix in range(n_cores):
        in_maps.append({
            "hT": hTd.astype(ml_dtypes.bfloat16),
            "waugT": waugT.astype(ml_dtypes.bfloat16),
            "gA": plan["gA"][cix],
            "gB": plan["gB"][cix],
            "gS": plan["gS"][cix],
            "gD": plan["gD"][cix],
            "onehot": onehot,
        })
    try:
        res = run_bass_kernel_spmd(nc, in_maps, list(range(n_cores)), trace=trace)
    except Exception:
        if not trace:
            raise
        res = run_bass_kernel_spmd(nc, in_maps, list(range(n_cores)), trace=False)

    out_full = np.zeros((plan["B"] * P, OUT_DIM), dtype=np.float32)
    for cix in range(n_cores):
        o = np.asarray(res.results[cix]["out"]).astype(np.float32)
        for j in range(plan["J"]):
            b = plan["assign"][cix, j]
            if b >= 0:
                out_full[b * P : (b + 1) * P] = o[j * P : (j + 1) * P]
    # un-permute columns: stored col = d*8+k -> [K, DK]
    out = out_full[:n_nodes].reshape(n_nodes, DK, K).transpose(0, 2, 1)
    return np.ascontiguousarray(out), res


def kernel(h, edge_src, edge_dst, W, attn):
    out, _ = run(h, edge_src, edge_dst, W, attn)
    return out
